# revision 5
# baseline (speedup 1.0000x reference)
"""Trainium2 Bass kernel for nn_MLP_Interpolate.

Reference computation (out_size=512, H=W=128, so exact 4x nearest upsample):
  out[b, :, 4k+r, 4l+s] = relu(x[b,:,k,l] @ W1[:64] + c[r,s]) @ W2 + b2
  c[r,s] = rel_y(r)*W1[64] + rel_x(s)*W1[65] + b1,  rel(t) = (2t-3)/4

Strategy (8 cores, shard = (batch, H-half)), v5:
  - F = W1c^T x on PE in bf16 (1 col/cycle) with a block-diagonal
    stationary: two 64-channel pixel groups per pass. x is prepacked on
    host into the per-tile block layout so each tile is ONE DMA.
  - F copied PSUM->SBUF as bf16 (ACT), then 16 bias+relu variants as bf16
    SBUF->SBUF split DVE/ACT.
  - pred on PE in bf16 with contiguous rhs streams (s-major column order).
    16 zero-padded [128,128] stationaries (FWL-eligible), one per (m,r),
    accumulate into one PSUM bank laid out p = 48*grp + 16*c + (4m+r).
  - One permuted-read PSUM->SBUF copy per bank on DVE restores output
    column order 4l+s, converting fp32 -> bf16 (output DMA'd as bf16 and
    upcast on host: halves SBUF->HBM traffic, which is limited to ~3 DMA
    engines / ~67 GB/s).
  - Out DMAs alternate sync/gpsimd queues; the final bank's drain is split
    into column halves so its DMA overlaps the second half's copy.
  - Startup: w1+crs merged into one DMA on the scalar HWDGE queue, x0 on
    sync, w2c compact on the vector HWDGE queue; dummy matmuls (fed by an
    early gpsimd memset) ramp the PE out of its 0.65GHz cold pstate.
"""

import numpy as np
import ml_dtypes

import concourse.bass as bass
import concourse.bacc as bacc
import concourse.mybir as mybir
import concourse.tile as tile
from concourse.bass_utils import run_bass_kernel_spmd

# Problem constants (hardcoded per contract)
B, C, H, W = 4, 64, 128, 128
OUT = 512
NF = 64  # n_feat
N_CORES = 8
ROWS_PER_CORE = H // 2          # 64 input rows per core
REL = np.array([-0.75, -0.25, 0.25, 0.75], dtype=np.float32)

# super-tiles as (input row0, n PSUM banks); 1 bank = 8 input rows
BLOCKS = [(0, 1), (8, 2), (24, 2), (40, 2), (56, 1)]
SEGS = [0, 4, 12, 20, 28]       # per-tile column offset in prepacked x

# relu variant -> engine (per tile): 12 on DVE, 4 on ACT (one per r-group,
# never the group's last variant so ACT is off the pred critical path)
ACT_V = {1, 6, 9, 14}

N_DUMMY = 6        # PE-warmup matmuls (512 cols each) before F(0)

_CACHE = {}


def _build_program():
    """Build + compile the SPMD Bass program once."""
    if "nc" in _CACHE:
        return _CACHE["nc"]

    fp32 = mybir.dt.float32
    bf16 = mybir.dt.bfloat16
    nc = bacc.Bacc("TRN2", target_bir_lowering=False, debug=False,
                   num_devices=N_CORES)

    x_d = nc.dram_tensor("xp", [128, 32, W], bf16, kind="ExternalInput")
    wc_d = nc.dram_tensor("w1d", [128, 128], bf16, kind="ExternalInput")
    crs_d = nc.dram_tensor("crsT", [128, 16], fp32, kind="ExternalInput")
    w2_d = nc.dram_tensor("w2c", [128, 16, 6], bf16, kind="ExternalInput")
    out_d = nc.dram_tensor("out", [3, 4 * ROWS_PER_CORE, OUT], bf16,
                           kind="ExternalOutput")

    with tile.TileContext(nc) as tc:
        with (
            tc.tile_pool(name="consts", bufs=1) as consts,
            tc.tile_pool(name="xin", bufs=2) as xin,
            tc.tile_pool(name="fbuf", bufs=2) as fbuf,
            tc.tile_pool(name="hbuf2", bufs=3) as hbuf2,
            tc.tile_pool(name="hbuf1", bufs=2) as hbuf1,
            tc.tile_pool(name="stage", bufs=4) as stage,
            tc.tile_pool(name="fpsum", bufs=2, space=bass.MemorySpace.PSUM) as fpsum,
            tc.tile_pool(name="ppsum", bufs=4, space=bass.MemorySpace.PSUM) as ppsum,
        ):
            wc_sb = consts.tile([128, 128], bf16)
            crs_sb = consts.tile([128, 16], fp32)
            w2_sb = consts.tile([128, 16, 128], bf16)
            w2c_sb = consts.tile([128, 16, 6], bf16)
            warm = consts.tile([1, 2], fp32)
            dm = consts.tile([128, 512], bf16)

            x_tiles = []
            f_tiles = []

            def load_x(ti):
                _, nb = BLOCKS[ti]
                seg = SEGS[ti]
                xt = xin.tile([128, 4 * nb, W], bf16, tag=f"xt{nb}")
                nc.sync.dma_start(xt[:, :, :], x_d[:, seg:seg + 4 * nb, :])
                x_tiles.append(xt)

            def feat_matmul(ti):
                _, nb = BLOCKS[ti]
                # fixed 2-bank tile regardless of nb so the pool stays at
                # 2 bufs x 2 banks; nb=1 tiles just use the first bank
                ft = fpsum.tile([128, 8, W], fp32, tag="ft")
                for half in range(nb):
                    nc.tensor.matmul(ft[:, 4 * half:4 * half + 4, :],
                                     wc_sb[:, 0:128],
                                     x_tiles[ti][:, 4 * half:4 * half + 4, :],
                                     start=True, stop=True)
                f_tiles.append(ft)

            def drain_bank(pt, r0, nb, ihb, last):
                """PSUM bank -> bf16 SBUF (permuted) -> out DMA(s)."""
                if not last:
                    st = stage.tile([128, OUT], bf16, tag="st")
                    nc.vector.tensor_copy(
                        st[0:96, :],
                        pt[0:96, :, :].rearrange("p s l -> p l s"))
                    for grp in range(2):
                        row0 = 4 * r0 + 16 * nb * grp + 16 * ihb
                        eng = nc.sync if grp == 0 else nc.gpsimd
                        eng.dma_start(
                            out_d[:, row0:row0 + 16, :],
                            st[48 * grp:48 * grp + 48, :])
                    return
                # final bank: split into column halves so the first half's
                # DMAs overlap the second half's copy
                for h in range(2):
                    sth = stage.tile([128, OUT // 2], bf16, tag="sth")
                    nc.vector.tensor_copy(
                        sth[0:96, :],
                        pt[0:96, :, 64 * h:64 * h + 64].rearrange(
                            "p s l -> p l s"))
                    for grp in range(2):
                        row0 = 4 * r0 + 16 * nb * grp + 16 * ihb
                        eng = nc.sync if grp == 0 else nc.scalar
                        eng.dma_start(
                            out_d[:, row0:row0 + 16,
                                  256 * h:256 * h + 256],
                            sth[48 * grp:48 * grp + 48, :])

            def tile_body(ti):
                r0, nb = BLOCKS[ti]
                ft = f_tiles[ti]
                # F: PSUM fp32 -> SBUF bf16, one pass on ACT
                fsb = fbuf.tile([128, 4 * nb, W], bf16, tag=f"fsb{nb}")
                nc.scalar.activation(fsb[:, :, :], ft[:, 0:4 * nb, :],
                                     mybir.ActivationFunctionType.Copy)

                # 16 bias+relu variants, bf16 SBUF -> SBUF (DVE/ACT only)
                hb = hbuf2 if nb == 2 else hbuf1
                ht = hb.tile([128, 16, 4 * nb, W], bf16, tag=f"ht{nb}")
                for v in range(16):
                    bias_ap = crs_sb[:, v:v + 1]
                    if v in ACT_V:
                        nc.scalar.activation(
                            ht[:, v, :, :], fsb[:, :, :],
                            mybir.ActivationFunctionType.Relu,
                            bias=bias_ap)
                    else:
                        nc.vector.tensor_scalar(
                            ht[:, v, :, :], fsb[:, :, :],
                            bias_ap, 0.0,
                            mybir.AluOpType.add, mybir.AluOpType.max)

                # pred: per bank, 16 matmuls accumulate into one PSUM bank.
                # contiguous rhs (stream order n' = 128s + l); partition
                # layout p = 48grp + 16c + (4m + r), 96 partitions used
                for ihb in range(nb):
                    pt = ppsum.tile([128, 4, 128], fp32, tag="pt")
                    for r in range(4):
                        for m in range(4):
                            i = 4 * ihb + m
                            nc.tensor.matmul(
                                pt[:, :, :], w2_sb[:, 4 * m + r, :],
                                ht[:, 4 * r:4 * r + 4, i, :],
                                start=(r == 0 and m == 0),
                                stop=(r == 3 and m == 3))
                    last = (ti == len(BLOCKS) - 1 and ihb == nb - 1)
                    drain_bank(pt, r0, nb, ihb, last)

            # startup: one merged w1+crs DMA on the scalar HWDGE queue,
            # x0 on sync, compact w2 on the vector HWDGE queue -- three
            # parallel queues so all constants land by ~t+10us
            nc.gpsimd.memset(dm[:], 0.0)
            nc.gpsimd.memset(warm[:, 0:1], 0.0)
            nc.scalar.dma_start(wc_sb[:], wc_d[:])
            nc.scalar.dma_start(crs_sb[:], crs_d[:])
            load_x(0)
            nc.gpsimd.dma_start(w2c_sb[:], w2_d[:])
            nc.vector.memset(w2_sb[:], 0.0)
            # prewarm the ACT function table so the ~1.3us ACT_TABLE_LOAD
            # overlaps the startup DMAs instead of delaying the first F copy
            nc.scalar.activation(warm[:, 1:2], warm[:, 0:1],
                                 mybir.ActivationFunctionType.Relu)
            # pre-ramp the PE while the startup DMA completions are in
            # flight: dummy matmuls on the memset tile push the PE out of
            # its 0.65GHz cold pstate before F(0) arrives
            dpt = ppsum.tile([128, 512], fp32, tag="pt")
            for _ in range(N_DUMMY):
                nc.tensor.matmul(dpt[:, :], dm[:, 0:128], dm[:, :],
                                 start=True, stop=True)
            load_x(1)
            # w2 stationaries arrive compact (24KB vs 512KB dense) and are
            # expanded on-chip: slice mr's live columns are mr + 16j for
            # j in 0..6, a uniform stride-16 scatter. Expansion ordered so
            # the r=0 stationaries (first pred matmuls) are ready first.
            for mr in (0, 4, 8, 12, 1, 5, 9, 13, 2, 6, 10, 14, 3, 7, 11, 15):
                nc.vector.tensor_copy(w2_sb[:, mr, mr:mr + 96:16],
                                      w2c_sb[:, mr, :])
            feat_matmul(0)
            feat_matmul(1)
            for ti in range(len(BLOCKS)):
                if ti + 2 < len(BLOCKS):
                    load_x(ti + 2)
                    feat_matmul(ti + 2)
                tile_body(ti)

    nc.compile()
    _CACHE["nc"] = nc
    return nc


def _prep_inputs(x, W1, b1, W2, b2):
    x = np.ascontiguousarray(np.asarray(x, dtype=np.float32))
    W1 = np.asarray(W1, dtype=np.float32)
    b1 = np.asarray(b1, dtype=np.float32)
    W2 = np.asarray(W2, dtype=np.float32)

    w1c = W1[:NF]                      # [64, 64]
    w1diag = np.zeros((128, 128), dtype=np.float32)
    w1diag[0:64, 0:64] = w1c
    w1diag[64:128, 64:128] = w1c

    # c[r,s] = rel[r]*W1[64] + rel[s]*W1[65] + b1 -> [16, 64]
    crs = (REL[:, None, None] * W1[NF][None, None, :]
           + REL[None, :, None] * W1[NF + 1][None, None, :]
           + b1[None, None, :]).reshape(16, NF)
    crsT = np.ascontiguousarray(
        np.concatenate([crs.T, crs.T], axis=0))    # [128, 16] fp32
    wc = np.ascontiguousarray(w1diag).astype(ml_dtypes.bfloat16)

    # pred stationaries, compact form: slice mr's live columns are
    # mr + 16j where j = 3*grp + c (full col = 48*grp + 16*c + mr);
    # expanded on-chip into zero-padded [128, 16, 128] FWL stationaries
    w2c = np.zeros((128, 16, 6), dtype=np.float32)
    for mr in range(16):
        for c in range(3):
            for grp in range(2):
                w2c[64 * grp:64 * grp + 64, mr, 3 * grp + c] = W2[:, c]
    w2c = w2c.astype(ml_dtypes.bfloat16)

    in_maps = []
    for c in range(N_CORES):
        b, half = c // 2, c % 2
        xs = x[b, :, half * ROWS_PER_CORE:(half + 1) * ROWS_PER_CORE, :]
        # prepack per-tile block-diag layout: for tile ti with rows
        # r0..r0+8nb, partition 64*blk + ch holds rows r0+4nb*blk ..,
        # so each tile load is a single contiguous-per-partition DMA
        xp = np.empty((128, 32, W), dtype=np.float32)
        for (r0, nb), seg in zip(BLOCKS, SEGS):
            blkv = xs[:, r0:r0 + 8 * nb, :].reshape(64, 2, 4 * nb, W)
            xp[:, seg:seg + 4 * nb, :] = \
                blkv.transpose(1, 0, 2, 3).reshape(128, 4 * nb, W)
        xp = np.ascontiguousarray(xp).astype(ml_dtypes.bfloat16)
        in_maps.append({"xp": xp, "w1d": wc, "crsT": crsT,
                        "w2c": w2c})
    return in_maps


def _gather(results, b2):
    full = np.empty((B, 3, OUT, OUT), dtype=np.float32)
    for c in range(N_CORES):
        b, half = c // 2, c % 2
        full[b, :, half * (OUT // 2):(half + 1) * (OUT // 2), :] = \
            results[c]["out"].astype(np.float32)
    b2 = np.asarray(b2, dtype=np.float32)
    if np.any(b2):
        full += b2.reshape(1, 3, 1, 1)
    return full


def run(trace=False, **inputs):
    nc = _build_program()
    in_maps = _prep_inputs(inputs["x"], inputs["W1"], inputs["b1"],
                           inputs["W2"], inputs["b2"])
    res = run_bass_kernel_spmd(nc, in_maps, list(range(N_CORES)), trace=trace)
    return _gather(res.results, inputs["b2"]), res


def kernel(**inputs):
    out, _ = run(trace=False, **inputs)
    return out
